# revision 1
# baseline (speedup 1.0000x reference)
"""ColorizationNet Trainium2 kernel (8 NeuronCores, SPMD, two phases).

Structure exploited: rows of the big FC input [4096, 32786] share an identical
x_conv prefix (32768 cols), so

    fc_in @ w1.T = x_conv @ w1[:, :32768].T  (one shared matvec, [304])
                 + [pos|chunks] @ w1[:, 32768:].T  ([4096,18] GEMM)

Sharding (core r of 8):
  - conv backbone row-sharded: core r produces the x_conv slice for pooled
    rows [4r, 4r+4) of every channel (halos via zero-padded input windows,
    out-of-image "phantom" rows masked to zero via activation scale).
  - shared matvec K-sharded to match (each core streams 1/8 of w1's big
    part, 5 MB, laid out so each SBUF partition's data is contiguous in
    DRAM).  Phase A outputs the 8 partials [304]; the host sums them
    (collectives are unavailable under the axon PJRT execution path).
  - phase B: patch FC sharded by patch row, core r handles patches
    [512r, 512(r+1)).

Convs use a banded-rows formulation: moving operand = input rows on SBUF
partitions (k = (row, c_in)), stationary = banded weight matrix with output
columns m = (s, rowpair, c_out) so the 2x2 maxpool's vertical pair is
partitions p / p+64 (one tensor_max) and the horizontal pair is a stride-2
free-dim pair.  Each layer's pooled activation is written by ScalarE
directly into the next layer's moving-window tiles (no DRAM round trips).
DMA count is minimized (one packed const tensor per phase) because each DMA
costs >1us of fixed sequencer/DGE overhead.
"""

import sys

for _p in ("/opt/trn_rl_repo",):
    if _p not in sys.path:
        sys.path.insert(0, _p)

import numpy as np
from contextlib import ExitStack

IMG = 256
CS = 4
G = 64
H1 = 304
H2 = 176
OUT = 48
NCORES = 8

# dtype knobs (mybir dtypes, set lazily in kernel()); float32 = exact,
# float32r = full-rate PE at reduced multiply precision
DT_MV = None
DT_FC = None
DT_CV = None

# phase-A packed const layout: [96 partitions, CA_W] fp32
#   s1  [18, 384]  at cols [0, 384)      (+ copy at rows 32..50 for block 2)
#   s2  [80, 384]  at cols [384, 768)
#   s3  [96, 384]  at cols [768, 1152)
#   mk1/bm1/mk2/bm2 [64, 3] at cols 1152/1155/1158/1161
#   bc3 [64, 1]    at col 1164
CA_W = 1165

# phase-B packed const layout: [128 partitions, CB_W] fp32
#   extrasT [18, 512] @0, w1eT [18, 304] @512, w2a/b [128, 176] @816/@992,
#   w2c [48, 176] @1168, w3a [128, 48] @1344, w3b [48, 48] @1392,
#   b2a [128, 1] @1440, b2b [48, 1] @1441, b3 [48, 1] @1442
CB_W = 1443


def _build_s1(c1_w):
    # [18, 3, 128]: rows i = in-row in window; cols m = s*64 + jp*8 + c
    s1 = np.zeros((18, 3, 128), np.float32)
    for dx in range(3):
        for s in range(2):
            for jp in range(8):
                j = 2 * jp + s
                for c in range(8):
                    m = s * 64 + jp * 8 + c
                    for dy in range(3):
                        s1[j + dy, dx, m] = c1_w[c, 0, dy, dx]
    return np.ascontiguousarray(s1.reshape(18, 3 * 128))


def _build_s2(c2_w):
    # [80, 3, 128]: rows k = delta*8 + ci (ci in 0..8); cols m = s*64+jp*16+co
    s2 = np.zeros((80, 3, 128), np.float32)
    for dx in range(3):
        for s in range(2):
            for jp in range(4):
                j2 = 2 * jp + s
                for co in range(16):
                    m = s * 64 + jp * 16 + co
                    for ci in range(8):
                        for dy in range(3):
                            s2[(j2 + dy) * 8 + ci, dx, m] = c2_w[co, ci, dy, dx]
    return np.ascontiguousarray(s2.reshape(80, 3 * 128))


def _build_s3(c3_w):
    # [96, 3, 128]: rows k = delta*16 + ci (ci in 0..16); cols m = s*64+jpp*32+co
    s3 = np.zeros((96, 3, 128), np.float32)
    for dx in range(3):
        for s in range(2):
            for jpp in range(2):
                j3 = 2 * jpp + s
                for co in range(32):
                    m = s * 64 + jpp * 32 + co
                    for ci in range(16):
                        for dy in range(3):
                            s3[(j3 + dy) * 16 + ci, dx, m] = c3_w[co, ci, dy, dx]
    return np.ascontiguousarray(s3.reshape(96, 3 * 128))


def _host_inputs(x, c1_w, c1_b, c2_w, c2_b, c3_w, c3_b, w1, b1, w2, b2, w3, b3):
    """Returns (in_maps_a, in_maps_b_partial, b1). Each phase-A map has
    'xs' [50,258], 'xs2' [34,258], 'ca' [96, CA_W], 'w1ps' [128, 9728].
    Each phase-B map has 'cb' [128, CB_W]; 'shc' [128, 3] is added after
    phase A."""
    x = np.asarray(x, np.float32).reshape(IMG, IMG)
    s1 = _build_s1(np.asarray(c1_w, np.float32))
    s2 = _build_s2(np.asarray(c2_w, np.float32))
    s3 = _build_s3(np.asarray(c3_w, np.float32))
    bc3 = np.tile(np.asarray(c3_b, np.float32), 2).reshape(64, 1)

    # phase-B packed consts (same for every core except extrasT)
    cb0 = np.zeros((128, CB_W), np.float32)
    w1eT = np.asarray(w1, np.float32)[:, 32768:].T  # [18, 304]
    w2T = np.asarray(w2, np.float32).T  # [304, 176]
    w3T = np.asarray(w3, np.float32).T  # [176, 48]
    cb0[0:18, 512:816] = w1eT
    cb0[0:128, 816:992] = w2T[0:128]
    cb0[0:128, 992:1168] = w2T[128:256]
    cb0[0:48, 1168:1344] = w2T[256:304]
    cb0[0:128, 1344:1392] = w3T[0:128]
    cb0[0:48, 1392:1440] = w3T[128:176]
    cb0[0:128, 1440:1441] = np.asarray(b2, np.float32)[0:128].reshape(128, 1)
    cb0[0:48, 1441:1442] = np.asarray(b2, np.float32)[128:176].reshape(48, 1)
    cb0[0:48, 1442:1443] = np.asarray(b3, np.float32).reshape(48, 1)

    w1bigT = np.ascontiguousarray(np.asarray(w1, np.float32)[:, :32768].T)  # [32768, 304]
    chunks = x.reshape(G, CS, G, CS).transpose(0, 2, 1, 3).reshape(G * G, CS * CS)
    pi = (np.arange(G * G) // G).astype(np.float32) * CS
    pj = (np.arange(G * G) % G).astype(np.float32) * CS

    P = np.arange(128)
    B = np.arange(32)
    c1b = np.asarray(c1_b, np.float32)
    c2b = np.asarray(c2_b, np.float32)

    maps_a, maps_b = [], []
    for r in range(NCORES):
        # xs: x rows [32r-7, 32r+43), cols padded by 1 each side
        xs = np.zeros((50, 258), np.float32)
        lo = 32 * r - 7
        hi = 32 * r + 43
        slo, shi = max(lo, 0), min(hi, IMG)
        xs[slo - lo : shi - lo, 1:257] = x[slo:shi, :]
        xs2 = np.ascontiguousarray(xs[16:50])  # [34, 258]

        ca = np.zeros((96, CA_W), np.float32)
        ca[0:18, 0:384] = s1
        ca[32:50, 0:384] = s1  # duplicate for the base-32 conv1 window
        # row-validity masks (zero out-of-image "phantom" pooled rows)
        for b in range(3):
            for jp in range(8):
                valid = 0 <= (16 * r - 3 + 8 * b + jp) < 128
                ca[jp * 8 : jp * 8 + 8, 384 + b] = 1.0 if valid else 0.0
                ca[jp * 8 : jp * 8 + 8, 387 + b] = c1b if valid else 0.0
            for jp in range(4):
                valid = 0 <= (8 * r - 1 + 4 * b + jp) < 64
                ca[jp * 16 : jp * 16 + 16, 390 + b] = 1.0 if valid else 0.0
                ca[jp * 16 : jp * 16 + 16, 393 + b] = c2b if valid else 0.0
        ca[0:64, 396:397] = bc3
        ca[0:80, 397:781] = s2
        ca[0:96, 781:1165] = s3

        # w1ps [128, 32*304]: w1ps[p, j*304+o] = w1[o, kglobal(p, j)],
        # kglobal = (p%32)*1024 + (4r + p//32)*32 + j
        kg = (P[None, :] % 32) * 1024 + (4 * r + P[None, :] // 32) * 32 + B[:, None]
        w1ps = np.ascontiguousarray(
            w1bigT[kg.ravel()].reshape(32, 128, 304).transpose(1, 0, 2).reshape(128, 32 * 304)
        )
        maps_a.append({"xs": xs, "xs2": xs2, "ca": ca, "w1ps": w1ps})

        cb = cb0.copy()
        sl = slice(512 * r, 512 * (r + 1))
        cb[0, 0:512] = pi[sl]
        cb[1, 0:512] = pj[sl]
        cb[2:18, 0:512] = chunks[sl].T
        maps_b.append({"cb": cb})
    return maps_a, maps_b, np.asarray(b1, np.float32)


def _mk_nc():
    import concourse.bacc as bacc

    # Bacc (not raw Bass): its compile() runs move_matmul_waits_to_ldweights /
    # generate_event_semaphores, required for the 1-wait-per-instruction
    # hardware constraint.
    return bacc.Bacc("TRN2", target_bir_lowering=False, debug=False, num_devices=NCORES)


def _build_phase_a(dt_mv, dt_cv):
    """Convs + sharded shared-matvec partial. Output: part [1, 304]."""
    import concourse.tile as tile
    from concourse import mybir

    f32 = mybir.dt.float32
    AF = mybir.ActivationFunctionType
    nc = _mk_nc()

    def din(name, shape):
        return nc.dram_tensor(name, list(shape), f32, kind="ExternalInput").ap()

    xs_d = din("xs", (50, 258))
    xs2_d = din("xs2", (34, 258))
    ca_d = din("ca", (96, CA_W))
    w1ps_d = din("w1ps", (128, 32 * 304))
    part_d = nc.dram_tensor("part", [1, 304], f32, kind="ExternalOutput").ap()

    with tile.TileContext(nc) as tc, ExitStack() as ctx:
        cpool = ctx.enter_context(tc.tile_pool(name="consts", bufs=1))
        spool = ctx.enter_context(tc.tile_pool(name="work", bufs=2))
        pconv = ctx.enter_context(tc.tile_pool(name="pconv", bufs=3, space="PSUM"))
        pmv = ctx.enter_context(tc.tile_pool(name="pmv", bufs=1, space="PSUM"))

        def _v(ap, dt):
            return ap if dt == f32 else ap.bitcast(dt)

        # warm the ScalarE activation-function table early (overlaps DMAs)
        scr = cpool.tile([1, 1], f32, tag="scr")
        nc.vector.memset(scr[:], 0.0)
        scr2 = cpool.tile([1, 1], f32, tag="scr2")
        nc.scalar.copy(scr2[:], scr[:])
        nc.scalar.activation(scr2[:], scr[:], AF.Relu)

        # small control inputs first so convs start immediately; conv1's
        # stationaries + masks (ca cols 0:397) land in their own small DMA
        ca_t = cpool.tile([96, CA_W], f32, tag="ca")
        nc.sync.dma_start(ca_t[:, 0:397], ca_d[:, 0:397])
        xs_t = cpool.tile([50, 258], f32, tag="xs")
        nc.sync.dma_start(xs_t[:], xs_d)
        xs2_t = cpool.tile([34, 258], f32, tag="xs2")
        nc.sync.dma_start(xs2_t[:], xs2_d)
        nc.sync.dma_start(ca_t[:, 397:CA_W], ca_d[:, 397:CA_W])

        # w1 stream: 4 chunk DMAs into one [128, 9728] tile, on the same SP
        # queue AFTER the control DMAs (queue FIFO keeps the small loads first)
        wst = cpool.tile([128, 32 * 304], f32, tag="w1s")
        CH = 4
        chw = 32 * 304 // CH
        for c in range(CH):
            nc.sync.dma_start(wst[:, c * chw : (c + 1) * chw], w1ps_d[:, c * chw : (c + 1) * chw])

        def s1ap(dx, base):  # stationary for conv1, at partition base 0 or 32
            return ca_t[base : base + 18, 128 * dx : 128 * (dx + 1)]

        def s2ap(dx):
            return ca_t[0:80, 397 + 128 * dx : 397 + 128 * (dx + 1)]

        def s3ap(dx):
            return ca_t[0:96, 781 + 128 * dx : 781 + 128 * (dx + 1)]

        mk1 = lambda b, n=64: ca_t[0:n, 384 + b : 385 + b]
        bm1 = lambda b, n=64: ca_t[0:n, 387 + b : 388 + b]
        mk2 = lambda b, n=64: ca_t[0:n, 390 + b : 391 + b]
        bm2 = lambda b, n=64: ca_t[0:n, 393 + b : 394 + b]
        bc3 = ca_t[0:64, 396:397]

        # next-layer moving-window tiles (built in place by ScalarE writes)
        m2 = [cpool.tile([80, 130], f32, tag=f"m2_{i}", name=f"m2_{i}") for i in range(3)]
        m3 = [cpool.tile([96, 66], f32, tag=f"m3_{i}", name=f"m3_{i}") for i in range(2)]
        xc_t = cpool.tile([128, 32], f32, tag="xc")
        for t in m2:
            nc.vector.memset(t[:], 0.0)
        for t in m3:
            nc.vector.memset(t[:], 0.0)

        def pool_to(ps, width):
            """psum [128, width] (m = (s, pair, c)) -> [64, width//2] max-pooled."""
            vtop = spool.tile([64, width], f32, tag=f"vt{width}")
            nc.scalar.copy(vtop[:], ps[0:64, :])
            v = spool.tile([64, width], f32, tag=f"v{width}")
            nc.vector.tensor_max(v[:], ps[64:128, :], vtop[:])
            vv = v[:].rearrange("p (x t) -> p x t", t=2)
            ph = spool.tile([64, width // 2], f32, tag=f"ph{width}")
            nc.vector.tensor_max(ph[:], vv[:, :, 0], vv[:, :, 1])
            return ph

        # ---- conv1: 3 blocks of 16 output rows -> M2 tiles
        win1 = [
            (xs_t[0:18, :], 0),
            (xs2_t[0:18, :], 0),
            (xs_t[32:50, :], 32),
        ]
        for b in range(3):
            rhs, base = win1[b]
            ps = pconv.tile([128, 256], f32, tag="cps")
            for dx in range(3):
                nc.tensor.matmul(
                    ps[:],
                    lhsT=_v(s1ap(dx, base), dt_cv),
                    rhs=_v(rhs[:, dx : dx + 256], dt_cv),
                    start=(dx == 0),
                    stop=(dx == 2),
                )
            ph = pool_to(ps, 256)  # [64, 128]: partition = jp*8+c, row = 8b+jp
            nc.scalar.activation(
                m2[b][0:64, 1:129], ph[:], AF.Relu, bias=bm1(b), scale=mk1(b)
            )
            if b >= 1:  # rows 8b, 8b+1 also tail rows 8..10 of previous window
                nc.scalar.activation(
                    m2[b - 1][64:80, 1:129],
                    ph[0:16, :],
                    AF.Relu,
                    bias=bm1(b, 16),
                    scale=mk1(b, 16),
                )

        # ---- conv2: 3 blocks of 8 output rows -> M3 tiles
        for b in range(3):
            ps = pconv.tile([128, 128], f32, tag="cps")
            for dx in range(3):
                nc.tensor.matmul(
                    ps[:],
                    lhsT=_v(s2ap(dx), dt_cv),
                    rhs=_v(m2[b][:, dx : dx + 128], dt_cv),
                    start=(dx == 0),
                    stop=(dx == 2),
                )
            ph = pool_to(ps, 128)  # [64, 64]: partition = jp'*16+co, row = 4b+jp'
            if b == 0:
                nc.scalar.activation(m3[0][0:64, 1:65], ph[:], AF.Relu, bias=bm2(0), scale=mk2(0))
            elif b == 1:
                nc.scalar.activation(m3[1][0:64, 1:65], ph[:], AF.Relu, bias=bm2(1), scale=mk2(1))
                nc.scalar.activation(
                    m3[0][64:96, 1:65], ph[0:32, :], AF.Relu, bias=bm2(1, 32), scale=mk2(1, 32)
                )
            else:
                nc.scalar.activation(
                    m3[1][64:96, 1:65], ph[0:32, :], AF.Relu, bias=bm2(2, 32), scale=mk2(2, 32)
                )

        # ---- conv3: 2 m-blocks of 4 output rows -> xc [128, 32]
        for g in range(2):
            ps = pconv.tile([128, 64], f32, tag="cps")
            for dx in range(3):
                nc.tensor.matmul(
                    ps[:],
                    lhsT=_v(s3ap(dx), dt_cv),
                    rhs=_v(m3[g][:, dx : dx + 64], dt_cv),
                    start=(dx == 0),
                    stop=(dx == 2),
                )
            ph = pool_to(ps, 64)  # [64, 32]
            nc.scalar.activation(xc_t[64 * g : 64 * g + 64, :], ph[:], AF.Relu, bias=bc3)

        # ---- shared matvec partial [1, 304]
        ps_mv = pmv.tile([1, 304], f32, tag="mv")
        for b in range(32):
            nc.tensor.matmul(
                ps_mv[:],
                lhsT=_v(xc_t[:, b : b + 1], dt_mv),
                rhs=_v(wst[:, 304 * b : 304 * (b + 1)], dt_mv),
                start=(b == 0),
                stop=(b == 31),
            )
        part_s = spool.tile([1, 304], f32, tag="part")
        nc.scalar.copy(part_s[:], ps_mv[:])
        nc.sync.dma_start(part_d, part_s[:])

    nc.compile()
    return nc


def _build_phase_b(dt_fc):
    """Patch FC for this core's 512 patches, given summed shared vector."""
    import concourse.tile as tile
    from concourse import mybir

    f32 = mybir.dt.float32
    AF = mybir.ActivationFunctionType
    nc = _mk_nc()

    cb_d = nc.dram_tensor("cb", [128, CB_W], f32, kind="ExternalInput").ap()
    shc_d = nc.dram_tensor("shc", [128, 3], f32, kind="ExternalInput").ap()
    yout_d = nc.dram_tensor("yout", [48, 512], f32, kind="ExternalOutput").ap()

    mblk = [(0, 128), (128, 128), (256, 48)]
    qblk = [(0, 128), (128, 48)]

    with tile.TileContext(nc) as tc, ExitStack() as ctx:
        cpool = ctx.enter_context(tc.tile_pool(name="consts", bufs=1))
        fpool = ctx.enter_context(tc.tile_pool(name="fc", bufs=1))
        pfc = ctx.enter_context(tc.tile_pool(name="pfc", bufs=1, space="PSUM"))
        phh = ctx.enter_context(tc.tile_pool(name="phh", bufs=3, space="PSUM"))

        def _v(ap, dt):
            return ap if dt == f32 else ap.bitcast(dt)

        # warm the ScalarE activation-function table early (overlaps DMAs)
        scr = cpool.tile([1, 1], f32, tag="scr")
        nc.vector.memset(scr[:], 0.0)
        scr2 = cpool.tile([1, 1], f32, tag="scr2")
        nc.scalar.activation(scr2[:], scr[:], AF.Relu)
        nc.scalar.activation(scr2[:], scr[:], AF.Sigmoid)

        cb = cpool.tile([128, CB_W], f32, tag="cb")
        nc.sync.dma_start(cb[:, 0:816], cb_d[:, 0:816])
        shc = cpool.tile([128, 3], f32, tag="shc")
        nc.scalar.dma_start(shc[:], shc_d)
        nc.scalar.dma_start(cb[:, 816:CB_W], cb_d[:, 816:CB_W])

        extrasT = cb[0:18, 0:512]
        w1eT = cb[0:18, 512:816]
        w2T_t = [cb[0:128, 816:992], cb[0:128, 992:1168], cb[0:48, 1168:1344]]
        w3T_t = [cb[0:128, 1344:1392], cb[0:48, 1392:1440]]
        b2c_t = [cb[0:128, 1440:1441], cb[0:48, 1441:1442]]
        b3c_t = cb[0:48, 1442:1443]
        sh_t = [shc[0:128, 0:1], shc[0:128, 1:2], shc[0:48, 2:3]]

        h1_t = []
        for i, (off, mb) in enumerate(mblk):
            ps_e = pfc.tile([mb, 512], f32, tag=f"pse{i}")
            nc.tensor.matmul(
                ps_e[:],
                lhsT=_v(w1eT[:, off : off + mb], dt_fc),
                rhs=_v(extrasT, dt_fc),
                start=True,
                stop=True,
            )
            h1 = fpool.tile([mb, 512], f32, tag=f"h1{i}")
            from concourse import mybir as _mb
            nc.vector.tensor_scalar(h1[:], ps_e[:], sh_t[i], 0.0, _mb.AluOpType.add, _mb.AluOpType.max)
            h1_t.append(h1)

        h2_t = []
        for q, (qoff, mq) in enumerate(qblk):
            ps_h = phh.tile([mq, 512], f32, tag="psh")
            for i, (off, mb) in enumerate(mblk):
                nc.tensor.matmul(
                    ps_h[:],
                    lhsT=_v(w2T_t[i][:, qoff : qoff + mq], dt_fc),
                    rhs=_v(h1_t[i][:], dt_fc),
                    start=(i == 0),
                    stop=(i == 2),
                )
            h2 = fpool.tile([mq, 512], f32, tag=f"h2{q}")
            nc.scalar.activation(h2[:], ps_h[:], AF.Relu, bias=b2c_t[q])
            h2_t.append(h2)

        ps_o = phh.tile([48, 512], f32, tag="psh")
        for q, (qoff, mq) in enumerate(qblk):
            nc.tensor.matmul(
                ps_o[:],
                lhsT=_v(w3T_t[q], dt_fc),
                rhs=_v(h2_t[q][:], dt_fc),
                start=(q == 0),
                stop=(q == 1),
            )
        outs = fpool.tile([48, 512], f32, tag="outs")
        nc.scalar.activation(outs[:], ps_o[:], AF.Sigmoid, bias=b3c_t)
        nc.sync.dma_start(yout_d, outs[:])

    nc.compile()
    return nc


def _shc_pack(sh):
    shc = np.zeros((128, 3), np.float32)
    shc[0:128, 0] = sh[0:128]
    shc[0:128, 1] = sh[128:256]
    shc[0:48, 2] = sh[256:304]
    return shc


def _run(maps_a, maps_b, b1, dt_mv, dt_fc, dt_cv, trace=False, trace_cores=None):
    from concourse.bass_utils import run_bass_kernel_spmd

    nca = _build_phase_a(dt_mv=dt_mv, dt_cv=dt_cv)
    res_a = run_bass_kernel_spmd(
        nca, maps_a, list(range(NCORES)), trace=trace, trace_cores=trace_cores
    )
    sh = np.sum([res_a.results[r]["part"][0] for r in range(NCORES)], axis=0) + b1
    shc = _shc_pack(sh)
    for mb in maps_b:
        mb["shc"] = shc
    ncb = _build_phase_b(dt_fc=dt_fc)
    res_b = run_bass_kernel_spmd(
        ncb, maps_b, list(range(NCORES)), trace=trace, trace_cores=trace_cores
    )
    full = np.empty((G * G, OUT), np.float32)
    for r in range(NCORES):
        full[512 * r : 512 * (r + 1), :] = res_b.results[r]["yout"].T
    return full.reshape(3, IMG, IMG), res_a, res_b


def kernel(**inputs):
    global DT_MV, DT_FC, DT_CV
    from concourse import mybir

    f32 = mybir.dt.float32
    if DT_MV is None:
        DT_MV = f32
    if DT_FC is None:
        DT_FC = f32
    if DT_CV is None:
        DT_CV = f32
    maps_a, maps_b, b1 = _host_inputs(**inputs)
    out, _, _ = _run(maps_a, maps_b, b1, DT_MV, DT_FC, DT_CV)
    return out


if __name__ == "__main__":
    import reference

    inp = {k: np.asarray(v) for k, v in reference.setup_inputs().items()}
    got = kernel(**inp)
    exp = np.asarray(reference.reference(**reference.setup_inputs()))
    err = np.abs(got - exp).max() / max(np.abs(exp).max(), 1e-9)
    print("Relative error:", err)



# revision 2
# speedup vs baseline: 1.4552x; 1.4552x over previous
"""ColorizationNet Trainium2 kernel (8 NeuronCores, SPMD, two phases).

Structure exploited: rows of the big FC input [4096, 32786] share an identical
x_conv prefix (32768 cols), so

    fc_in @ w1.T = x_conv @ w1[:, :32768].T  (one shared matvec, [304])
                 + [pos|chunks] @ w1[:, 32768:].T  ([4096,18] GEMM)

Sharding (core r of 8):
  - conv backbone row-sharded: core r produces the x_conv slice for pooled
    rows [4r, 4r+4) of every channel (halos via zero-padded input windows,
    out-of-image "phantom" rows masked to zero via activation scale).
  - shared matvec K-sharded to match (each core streams 1/8 of w1's big
    part, fp16, laid out so each SBUF partition's data is contiguous in
    DRAM).  Phase A outputs the 8 partials [304]; the host sums them
    (collectives work under this path but carry ~50us of cross-core launch
    skew + protocol cost, measured — host sum between phases is free).
  - phase B: patch FC sharded by patch row, core r handles patches
    [512r, 512(r+1)).

All matmul operands are fp16 (PSUM accumulation stays fp32): fp32 matmuls
cost 4 PE cycles/row vs 1 for fp16, and fp16 also halves the w1 stream
(2.5 MB/core).  Tolerance is 2e-2; measured error ~1e-3.

Convs use a banded-rows formulation: moving operand = input rows on SBUF
partitions (k = (row, c_in)), stationary = banded weight matrix with output
columns m = (s, rowpair, c_out) so the 2x2 maxpool's vertical pair is
partitions p / p+64 (one tensor_max) and the horizontal pair is a stride-2
free-dim pair.  Each layer's pooled activation is written by ScalarE
directly into the next layer's moving-window tiles (no DRAM round trips).
The w1 stream is split across the two HWDGE queues (Sync + Scalar).
"""

import sys

for _p in ("/opt/trn_rl_repo",):
    if _p not in sys.path:
        sys.path.insert(0, _p)

import numpy as np
from contextlib import ExitStack

IMG = 256
CS = 4
G = 64
H1 = 304
H2 = 176
OUT = 48
NCORES = 8

# fp16 const layouts
CS16_W = 1152  # s1 [18,384]@0 (+dup rows 32:50), s2 [80,384]@384, s3 [96,384]@768
CA32_W = 13    # mk1[0:3] bm1[3:6] mk2[6:9] bm2[9:12] bc3[12]
CB16_W = 1440  # extrasT[18,512]@0, w1eT[18,304]@512, w2a/b[128,176]@816/@992,
               # w2c[48,176]@1168, w3a[128,48]@1344, w3b[48,48]@1392
CB32_W = 3     # b2a[128]@0, b2b[48]@1, b3[48]@2


def _build_s1(c1_w):
    # [18, 3, 128]: rows i = in-row in window; cols m = s*64 + jp*8 + c
    s1 = np.zeros((18, 3, 128), np.float32)
    for dx in range(3):
        for s in range(2):
            for jp in range(8):
                j = 2 * jp + s
                for c in range(8):
                    m = s * 64 + jp * 8 + c
                    for dy in range(3):
                        s1[j + dy, dx, m] = c1_w[c, 0, dy, dx]
    return np.ascontiguousarray(s1.reshape(18, 3 * 128))


def _build_s2(c2_w):
    # [80, 3, 128]: rows k = delta*8 + ci (ci in 0..8); cols m = s*64+jp*16+co
    s2 = np.zeros((80, 3, 128), np.float32)
    for dx in range(3):
        for s in range(2):
            for jp in range(4):
                j2 = 2 * jp + s
                for co in range(16):
                    m = s * 64 + jp * 16 + co
                    for ci in range(8):
                        for dy in range(3):
                            s2[(j2 + dy) * 8 + ci, dx, m] = c2_w[co, ci, dy, dx]
    return np.ascontiguousarray(s2.reshape(80, 3 * 128))


def _build_s3(c3_w):
    # [96, 3, 128]: rows k = delta*16 + ci (ci in 0..16); cols m = s*64+jpp*32+co
    s3 = np.zeros((96, 3, 128), np.float32)
    for dx in range(3):
        for s in range(2):
            for jpp in range(2):
                j3 = 2 * jpp + s
                for co in range(32):
                    m = s * 64 + jpp * 32 + co
                    for ci in range(16):
                        for dy in range(3):
                            s3[(j3 + dy) * 16 + ci, dx, m] = c3_w[co, ci, dy, dx]
    return np.ascontiguousarray(s3.reshape(96, 3 * 128))


def _host_inputs(x, c1_w, c1_b, c2_w, c2_b, c3_w, c3_b, w1, b1, w2, b2, w3, b3):
    """Returns (in_maps_a, in_maps_b_partial, b1). Phase-A map: 'xs' [50,258]f16,
    'xs2' [34,258]f16, 'cs16' [96,CS16_W]f16, 'ca32' [64,CA32_W]f32,
    'w1ps' [128, 9728]f16.  Phase-B map: 'cb16' [128,CB16_W]f16,
    'cb32' [128,CB32_W]f32; 'shc' [128,3]f32 added after phase A."""
    f16 = np.float16
    x = np.asarray(x, np.float32).reshape(IMG, IMG)
    s1 = _build_s1(np.asarray(c1_w, np.float32))
    s2 = _build_s2(np.asarray(c2_w, np.float32))
    s3 = _build_s3(np.asarray(c3_w, np.float32))
    bc3 = np.tile(np.asarray(c3_b, np.float32), 2).reshape(64, 1)

    cs16 = np.zeros((96, CS16_W), f16)
    cs16[0:18, 0:384] = s1
    cs16[32:50, 0:384] = s1  # duplicate for the base-32 conv1 window
    cs16[0:80, 384:768] = s2
    cs16[0:96, 768:1152] = s3

    # phase-B packed consts (same for every core except extrasT)
    cb0 = np.zeros((128, CB16_W), f16)
    w1eT = np.asarray(w1, np.float32)[:, 32768:].T  # [18, 304]
    w2T = np.asarray(w2, np.float32).T  # [304, 176]
    w3T = np.asarray(w3, np.float32).T  # [176, 48]
    cb0[0:18, 512:816] = w1eT
    cb0[0:128, 816:992] = w2T[0:128]
    cb0[0:128, 992:1168] = w2T[128:256]
    cb0[0:48, 1168:1344] = w2T[256:304]
    cb0[0:128, 1344:1392] = w3T[0:128]
    cb0[0:48, 1392:1440] = w3T[128:176]
    cb32 = np.zeros((128, CB32_W), np.float32)
    cb32[0:128, 0] = np.asarray(b2, np.float32)[0:128]
    cb32[0:48, 1] = np.asarray(b2, np.float32)[128:176]
    cb32[0:48, 2] = np.asarray(b3, np.float32)

    w1bigT = np.ascontiguousarray(np.asarray(w1, np.float32)[:, :32768].T)  # [32768, 304]
    chunks = x.reshape(G, CS, G, CS).transpose(0, 2, 1, 3).reshape(G * G, CS * CS)
    pi = (np.arange(G * G) // G).astype(np.float32) * CS
    pj = (np.arange(G * G) % G).astype(np.float32) * CS

    P = np.arange(128)
    B = np.arange(32)
    c1b = np.asarray(c1_b, np.float32)
    c2b = np.asarray(c2_b, np.float32)

    maps_a, maps_b = [], []
    for r in range(NCORES):
        # xs: x rows [32r-7, 32r+43), cols padded by 1 each side
        xs = np.zeros((50, 258), f16)
        lo = 32 * r - 7
        hi = 32 * r + 43
        slo, shi = max(lo, 0), min(hi, IMG)
        xs[slo - lo : shi - lo, 1:257] = x[slo:shi, :]
        xs2 = np.ascontiguousarray(xs[16:50])  # [34, 258]

        ca32 = np.zeros((64, CA32_W), np.float32)
        # row-validity masks (zero out-of-image "phantom" pooled rows)
        for b in range(3):
            for jp in range(8):
                valid = 0 <= (16 * r - 3 + 8 * b + jp) < 128
                ca32[jp * 8 : jp * 8 + 8, 0 + b] = 1.0 if valid else 0.0
                ca32[jp * 8 : jp * 8 + 8, 3 + b] = c1b if valid else 0.0
            for jp in range(4):
                valid = 0 <= (8 * r - 1 + 4 * b + jp) < 64
                ca32[jp * 16 : jp * 16 + 16, 6 + b] = 1.0 if valid else 0.0
                ca32[jp * 16 : jp * 16 + 16, 9 + b] = c2b if valid else 0.0
        ca32[0:64, 12:13] = bc3

        # w1ps [128, 32*304]: w1ps[p, j*304+o] = w1[o, kglobal(p, j)],
        # kglobal = (p%32)*1024 + (4r + p//32)*32 + j
        kg = (P[None, :] % 32) * 1024 + (4 * r + P[None, :] // 32) * 32 + B[:, None]
        w1ps = np.ascontiguousarray(
            w1bigT[kg.ravel()]
            .reshape(32, 128, 304)
            .transpose(1, 0, 2)
            .reshape(128, 32 * 304)
            .astype(f16)
        )
        maps_a.append({"xs": xs, "xs2": xs2, "cs16": cs16, "ca32": ca32, "w1ps": w1ps})

        cb = cb0.copy()
        sl = slice(512 * r, 512 * (r + 1))
        cb[0, 0:512] = pi[sl]
        cb[1, 0:512] = pj[sl]
        cb[2:18, 0:512] = chunks[sl].T
        maps_b.append({"cb16": cb, "cb32": cb32})
    return maps_a, maps_b, np.asarray(b1, np.float32)


def _mk_nc():
    import concourse.bacc as bacc

    # Bacc (not raw Bass): its compile() runs move_matmul_waits_to_ldweights /
    # generate_event_semaphores, required for the 1-wait-per-instruction
    # hardware constraint.
    return bacc.Bacc("TRN2", target_bir_lowering=False, debug=False, num_devices=NCORES)


def _build_phase_a():
    """Convs + sharded shared-matvec partial. Output: part [1, 304]."""
    import concourse.tile as tile
    from concourse import mybir

    f32 = mybir.dt.float32
    f16 = mybir.dt.float16
    AF = mybir.ActivationFunctionType
    nc = _mk_nc()

    def din(name, shape, dt):
        return nc.dram_tensor(name, list(shape), dt, kind="ExternalInput").ap()

    xs_d = din("xs", (50, 258), f16)
    xs2_d = din("xs2", (34, 258), f16)
    cs16_d = din("cs16", (96, CS16_W), f16)
    ca32_d = din("ca32", (64, CA32_W), f32)
    w1ps_d = din("w1ps", (128, 32 * 304), f16)
    part_d = nc.dram_tensor("part", [1, 304], f32, kind="ExternalOutput").ap()

    with tile.TileContext(nc) as tc, ExitStack() as ctx:
        cpool = ctx.enter_context(tc.tile_pool(name="consts", bufs=1))
        spool = ctx.enter_context(tc.tile_pool(name="work", bufs=2))
        pconv = ctx.enter_context(tc.tile_pool(name="pconv", bufs=3, space="PSUM"))
        pmv = ctx.enter_context(tc.tile_pool(name="pmv", bufs=1, space="PSUM"))

        # warm the ScalarE activation-function table early (overlaps DMAs)
        scr = cpool.tile([1, 1], f32, tag="scr")
        nc.vector.memset(scr[:], 0.0)
        scr2 = cpool.tile([1, 1], f32, tag="scr2")
        nc.scalar.copy(scr2[:], scr[:])
        nc.scalar.activation(scr2[:], scr[:], AF.Relu)

        # small control inputs first so convs start immediately
        ca32_t = cpool.tile([64, CA32_W], f32, tag="ca32")
        nc.sync.dma_start(ca32_t[:], ca32_d)
        cs16_t = cpool.tile([96, CS16_W], f16, tag="cs16")
        nc.sync.dma_start(cs16_t[:], cs16_d)
        xs_t = cpool.tile([50, 258], f16, tag="xs")
        nc.sync.dma_start(xs_t[:], xs_d)
        xs2_t = cpool.tile([34, 258], f16, tag="xs2")
        nc.sync.dma_start(xs2_t[:], xs2_d)

        # w1 stream: 4 chunk DMAs into one [128, 9728] fp16 tile, split
        # across the two HWDGE queues (Sync gets 0,2; Scalar gets 1,3)
        wst = cpool.tile([128, 32 * 304], f16, tag="w1s")
        CH = 4
        chw = 32 * 304 // CH
        for c in range(CH):
            eng = nc.sync if c % 2 == 0 else nc.scalar
            eng.dma_start(wst[:, c * chw : (c + 1) * chw], w1ps_d[:, c * chw : (c + 1) * chw])

        def s1ap(dx, base):  # stationary for conv1, at partition base 0 or 32
            return cs16_t[base : base + 18, 128 * dx : 128 * (dx + 1)]

        def s2ap(dx):
            return cs16_t[0:80, 384 + 128 * dx : 384 + 128 * (dx + 1)]

        def s3ap(dx):
            return cs16_t[0:96, 768 + 128 * dx : 768 + 128 * (dx + 1)]

        mk1 = lambda b, n=64: ca32_t[0:n, 0 + b : 1 + b]
        bm1 = lambda b, n=64: ca32_t[0:n, 3 + b : 4 + b]
        mk2 = lambda b, n=64: ca32_t[0:n, 6 + b : 7 + b]
        bm2 = lambda b, n=64: ca32_t[0:n, 9 + b : 10 + b]
        bc3 = ca32_t[0:64, 12:13]

        # next-layer moving-window tiles (built in place by ScalarE writes)
        m2 = [cpool.tile([80, 130], f16, tag=f"m2_{i}", name=f"m2_{i}") for i in range(3)]
        m3 = [cpool.tile([96, 66], f16, tag=f"m3_{i}", name=f"m3_{i}") for i in range(2)]
        xc_t = cpool.tile([128, 32], f16, tag="xc")
        for t in m2:
            nc.vector.memset(t[:], 0.0)
        for t in m3:
            nc.vector.memset(t[:], 0.0)

        def pool_to(ps, width):
            """psum [128, width] (m = (s, pair, c)) -> [64, width//2] max-pooled."""
            vtop = spool.tile([64, width], f32, tag=f"vt{width}")
            nc.scalar.copy(vtop[:], ps[0:64, :])
            v = spool.tile([64, width], f32, tag=f"v{width}")
            nc.vector.tensor_max(v[:], ps[64:128, :], vtop[:])
            vv = v[:].rearrange("p (x t) -> p x t", t=2)
            ph = spool.tile([64, width // 2], f32, tag=f"ph{width}")
            nc.vector.tensor_max(ph[:], vv[:, :, 0], vv[:, :, 1])
            return ph

        # ---- conv1: 3 blocks of 16 output rows -> M2 tiles
        win1 = [
            (xs_t[0:18, :], 0),
            (xs2_t[0:18, :], 0),
            (xs_t[32:50, :], 32),
        ]
        for b in range(3):
            rhs, base = win1[b]
            ps = pconv.tile([128, 256], f32, tag="cps")
            for dx in range(3):
                nc.tensor.matmul(
                    ps[:],
                    lhsT=s1ap(dx, base),
                    rhs=rhs[:, dx : dx + 256],
                    start=(dx == 0),
                    stop=(dx == 2),
                )
            ph = pool_to(ps, 256)  # [64, 128]: partition = jp*8+c, row = 8b+jp
            nc.scalar.activation(
                m2[b][0:64, 1:129], ph[:], AF.Relu, bias=bm1(b), scale=mk1(b)
            )
            if b >= 1:  # rows 8b, 8b+1 also tail rows 8..10 of previous window
                nc.scalar.activation(
                    m2[b - 1][64:80, 1:129],
                    ph[0:16, :],
                    AF.Relu,
                    bias=bm1(b, 16),
                    scale=mk1(b, 16),
                )

        # ---- conv2: 3 blocks of 8 output rows -> M3 tiles
        for b in range(3):
            ps = pconv.tile([128, 128], f32, tag="cps")
            for dx in range(3):
                nc.tensor.matmul(
                    ps[:],
                    lhsT=s2ap(dx),
                    rhs=m2[b][:, dx : dx + 128],
                    start=(dx == 0),
                    stop=(dx == 2),
                )
            ph = pool_to(ps, 128)  # [64, 64]: partition = jp'*16+co, row = 4b+jp'
            if b == 0:
                nc.scalar.activation(m3[0][0:64, 1:65], ph[:], AF.Relu, bias=bm2(0), scale=mk2(0))
            elif b == 1:
                nc.scalar.activation(m3[1][0:64, 1:65], ph[:], AF.Relu, bias=bm2(1), scale=mk2(1))
                nc.scalar.activation(
                    m3[0][64:96, 1:65], ph[0:32, :], AF.Relu, bias=bm2(1, 32), scale=mk2(1, 32)
                )
            else:
                nc.scalar.activation(
                    m3[1][64:96, 1:65], ph[0:32, :], AF.Relu, bias=bm2(2, 32), scale=mk2(2, 32)
                )

        # ---- conv3: 2 m-blocks of 4 output rows -> xc [128, 32]
        for g in range(2):
            ps = pconv.tile([128, 64], f32, tag="cps")
            for dx in range(3):
                nc.tensor.matmul(
                    ps[:],
                    lhsT=s3ap(dx),
                    rhs=m3[g][:, dx : dx + 64],
                    start=(dx == 0),
                    stop=(dx == 2),
                )
            ph = pool_to(ps, 64)  # [64, 32]
            nc.scalar.activation(xc_t[64 * g : 64 * g + 64, :], ph[:], AF.Relu, bias=bc3)

        # ---- shared matvec partial [1, 304]
        ps_mv = pmv.tile([1, 304], f32, tag="mv")
        for b in range(32):
            nc.tensor.matmul(
                ps_mv[:],
                lhsT=xc_t[:, b : b + 1],
                rhs=wst[:, 304 * b : 304 * (b + 1)],
                start=(b == 0),
                stop=(b == 31),
            )
        part_s = spool.tile([1, 304], f32, tag="part")
        nc.scalar.copy(part_s[:], ps_mv[:])
        nc.sync.dma_start(part_d, part_s[:])

    nc.compile()
    return nc


def _build_phase_b():
    """Patch FC for this core's 512 patches, given summed shared vector."""
    import concourse.tile as tile
    from concourse import mybir

    f32 = mybir.dt.float32
    f16 = mybir.dt.float16
    AF = mybir.ActivationFunctionType
    nc = _mk_nc()

    cb16_d = nc.dram_tensor("cb16", [128, CB16_W], f16, kind="ExternalInput").ap()
    cb32_d = nc.dram_tensor("cb32", [128, CB32_W], f32, kind="ExternalInput").ap()
    shc_d = nc.dram_tensor("shc", [128, 3], f32, kind="ExternalInput").ap()
    yout_d = nc.dram_tensor("yout", [48, 512], f16, kind="ExternalOutput").ap()

    mblk = [(0, 128), (128, 128), (256, 48)]
    qblk = [(0, 128), (128, 48)]

    with tile.TileContext(nc) as tc, ExitStack() as ctx:
        cpool = ctx.enter_context(tc.tile_pool(name="consts", bufs=1))
        fpool = ctx.enter_context(tc.tile_pool(name="fc", bufs=1))
        pfc = ctx.enter_context(tc.tile_pool(name="pfc", bufs=1, space="PSUM"))
        phh = ctx.enter_context(tc.tile_pool(name="phh", bufs=3, space="PSUM"))

        # warm the ScalarE activation-function table early (overlaps DMAs)
        scr = cpool.tile([1, 1], f32, tag="scr")
        nc.vector.memset(scr[:], 0.0)
        scr2 = cpool.tile([1, 1], f32, tag="scr2")
        nc.scalar.activation(scr2[:], scr[:], AF.Relu)
        nc.scalar.activation(scr2[:], scr[:], AF.Sigmoid)

        cb = cpool.tile([128, CB16_W], f16, tag="cb16")
        nc.sync.dma_start(cb[:, 0:816], cb16_d[:, 0:816])
        cb32 = cpool.tile([128, CB32_W], f32, tag="cb32")
        nc.scalar.dma_start(cb32[:], cb32_d)
        shc = cpool.tile([128, 3], f32, tag="shc")
        nc.scalar.dma_start(shc[:], shc_d)
        nc.scalar.dma_start(cb[:, 816:CB16_W], cb16_d[:, 816:CB16_W])

        extrasT = cb[0:18, 0:512]
        w1eT = cb[0:18, 512:816]
        w2T_t = [cb[0:128, 816:992], cb[0:128, 992:1168], cb[0:48, 1168:1344]]
        w3T_t = [cb[0:128, 1344:1392], cb[0:48, 1392:1440]]
        b2c_t = [cb32[0:128, 0:1], cb32[0:48, 1:2]]
        b3c_t = cb32[0:48, 2:3]
        sh_t = [shc[0:128, 0:1], shc[0:128, 1:2], shc[0:48, 2:3]]

        from concourse import mybir as _mb

        h1_t = []
        for i, (off, mb) in enumerate(mblk):
            ps_e = pfc.tile([mb, 512], f32, tag=f"pse{i}")
            nc.tensor.matmul(
                ps_e[:],
                lhsT=w1eT[:, off : off + mb],
                rhs=extrasT,
                start=True,
                stop=True,
            )
            h1 = fpool.tile([mb, 512], f16, tag=f"h1{i}")
            nc.vector.tensor_scalar(
                h1[:], ps_e[:], sh_t[i], 0.0, _mb.AluOpType.add, _mb.AluOpType.max
            )
            h1_t.append(h1)

        h2_t = []
        for q, (qoff, mq) in enumerate(qblk):
            ps_h = phh.tile([mq, 512], f32, tag="psh")
            for i, (off, mb) in enumerate(mblk):
                nc.tensor.matmul(
                    ps_h[:],
                    lhsT=w2T_t[i][:, qoff : qoff + mq],
                    rhs=h1_t[i][:],
                    start=(i == 0),
                    stop=(i == 2),
                )
            h2 = fpool.tile([mq, 512], f16, tag=f"h2{q}")
            nc.scalar.activation(h2[:], ps_h[:], AF.Relu, bias=b2c_t[q])
            h2_t.append(h2)

        ps_o = phh.tile([48, 512], f32, tag="psh")
        for q, (qoff, mq) in enumerate(qblk):
            nc.tensor.matmul(
                ps_o[:],
                lhsT=w3T_t[q],
                rhs=h2_t[q][:],
                start=(q == 0),
                stop=(q == 1),
            )
        outs = fpool.tile([48, 512], f16, tag="outs")
        nc.scalar.activation(outs[:], ps_o[:], AF.Sigmoid, bias=b3c_t)
        nc.sync.dma_start(yout_d, outs[:])

    nc.compile()
    return nc


def _shc_pack(sh):
    shc = np.zeros((128, 3), np.float32)
    shc[0:128, 0] = sh[0:128]
    shc[0:128, 1] = sh[128:256]
    shc[0:48, 2] = sh[256:304]
    return shc


def _run(maps_a, maps_b, b1, trace=False, trace_cores=None):
    from concourse.bass_utils import run_bass_kernel_spmd

    nca = _build_phase_a()
    res_a = run_bass_kernel_spmd(
        nca, maps_a, list(range(NCORES)), trace=trace, trace_cores=trace_cores
    )
    sh = np.sum([res_a.results[r]["part"][0] for r in range(NCORES)], axis=0) + b1
    shc = _shc_pack(sh)
    for mb in maps_b:
        mb["shc"] = shc
    ncb = _build_phase_b()
    res_b = run_bass_kernel_spmd(
        ncb, maps_b, list(range(NCORES)), trace=trace, trace_cores=trace_cores
    )
    full = np.empty((G * G, OUT), np.float32)
    for r in range(NCORES):
        full[512 * r : 512 * (r + 1), :] = res_b.results[r]["yout"].T.astype(np.float32)
    return full.reshape(3, IMG, IMG), res_a, res_b


def kernel(**inputs):
    maps_a, maps_b, b1 = _host_inputs(**inputs)
    out, _, _ = _run(maps_a, maps_b, b1)
    return out


if __name__ == "__main__":
    import reference

    inp = {k: np.asarray(v) for k, v in reference.setup_inputs().items()}
    got = kernel(**inp)
    exp = np.asarray(reference.reference(**reference.setup_inputs()))
    err = np.abs(got - exp).max() / max(np.abs(exp).max(), 1e-9)
    print("Relative error:", err)


# revision 4
# speedup vs baseline: 1.5677x; 1.0773x over previous
"""ColorizationNet Trainium2 kernel (8 NeuronCores, SPMD, two phases).

Structure exploited: rows of the big FC input [4096, 32786] share an identical
x_conv prefix (32768 cols), so

    fc_in @ w1.T = x_conv @ w1[:, :32768].T  (one shared matvec, [304])
                 + [pos|chunks] @ w1[:, 32768:].T  ([4096,18] GEMM)

Sharding (core r of 8):
  - conv backbone row-sharded: core r produces the x_conv slice for pooled
    rows [4r, 4r+4) of every channel (halos via zero-padded input windows,
    out-of-image "phantom" rows masked to zero via activation scale).
  - shared matvec K-sharded to match (each core streams 1/8 of w1's big
    part, fp16).  Phase A outputs the 8 partials [304]; the host sums them
    (on-device collectives work here but carry ~50us of cross-core launch
    skew, measured — the host sum between launches is free).
  - phase B: patch FC sharded by patch row, core r handles patches
    [512r, 512(r+1)).

Perf structure (measured on trn2):
  - all matmul operands fp16 (fp32 costs 4 PE cycles/row vs 1; fp16 also
    halves the w1 stream to 2.5 MB/core).  PSUM accumulation stays fp32;
    tolerance is 2e-2, measured error ~6e-4.
  - each dma_start costs ~0.6-0.9us issue + queue latency, so all small
    consts ship as ONE packed fp16 tensor per phase (fp32 mask/bias
    regions ride along bit-cast into fp16 column pairs).
  - the w1 stream is split across the two HWDGE queues (Sync+Scalar);
    a single queue sustains ~390 GB/s only when nothing contends.
  - the PE runs ~2.3x slow until it has been busy ~3us (p-state ramp), so
    both phases issue dummy warmup matmuls during the dead DMA-wait window.
"""

import sys

for _p in ("/opt/trn_rl_repo",):
    if _p not in sys.path:
        sys.path.insert(0, _p)

import numpy as np
from contextlib import ExitStack

IMG = 256
CS = 4
G = 64
H1 = 304
H2 = 176
OUT = 48
NCORES = 8

# phase-A packed const layout, [96, CC_W] fp16:
#   xs [50,258]@0, xs2(=xs rows 16:34) [18,258]@258, s1 [18,384]@516 (+dup
#   rows 32:50), s2 [80,384]@900, s3 [96,384]@1284, ca32 (13 fp32 as 26
#   fp16 cols) [64,26]@1668:  mk1[0:3] bm1[3:6] mk2[6:9] bm2[9:12] bc3[12]
CC_W = 1694
# phase-B fp16 consts, [128, CB16_W]:
#   extrasT [18,512]@0, w1eT [18,304]@512, w2a/b [128,176]@816/@992,
#   w2c [48,176]@1168, w3a [128,48]@1344, w3b [48,48]@1392
CB16_W = 1440
# phase-B fp32 tensor [128, 6]: sh0 sh1 sh2 b2a b2b b3  (sh = summed shared
# vector, packed column-wise after phase A)
SH_W = 6


def _build_s1(c1_w):
    # [18, 3, 128]: rows i = in-row in window; cols m = s*64 + jp*8 + c
    s1 = np.zeros((18, 3, 128), np.float32)
    for dx in range(3):
        for s in range(2):
            for jp in range(8):
                j = 2 * jp + s
                for c in range(8):
                    m = s * 64 + jp * 8 + c
                    for dy in range(3):
                        s1[j + dy, dx, m] = c1_w[c, 0, dy, dx]
    return np.ascontiguousarray(s1.reshape(18, 3 * 128))


def _build_s2(c2_w):
    # [80, 3, 128]: rows k = delta*8 + ci (ci in 0..8); cols m = s*64+jp*16+co
    s2 = np.zeros((80, 3, 128), np.float32)
    for dx in range(3):
        for s in range(2):
            for jp in range(4):
                j2 = 2 * jp + s
                for co in range(16):
                    m = s * 64 + jp * 16 + co
                    for ci in range(8):
                        for dy in range(3):
                            s2[(j2 + dy) * 8 + ci, dx, m] = c2_w[co, ci, dy, dx]
    return np.ascontiguousarray(s2.reshape(80, 3 * 128))


def _build_s3(c3_w):
    # [96, 3, 128]: rows k = delta*16 + ci (ci in 0..16); cols m = s*64+jpp*32+co
    s3 = np.zeros((96, 3, 128), np.float32)
    for dx in range(3):
        for s in range(2):
            for jpp in range(2):
                j3 = 2 * jpp + s
                for co in range(32):
                    m = s * 64 + jpp * 32 + co
                    for ci in range(16):
                        for dy in range(3):
                            s3[(j3 + dy) * 16 + ci, dx, m] = c3_w[co, ci, dy, dx]
    return np.ascontiguousarray(s3.reshape(96, 3 * 128))


def _host_inputs(x, c1_w, c1_b, c2_w, c2_b, c3_w, c3_b, w1, b1, w2, b2, w3, b3):
    """Returns (in_maps_a, in_maps_b_partial, b1, shc0).  Phase-A map:
    'cc' [96,CC_W]f16, 'w1ps' [128,9728]f16.  Phase-B map: 'cb16'
    [128,CB16_W]f16; 'shc' [128,SH_W]f32 (shc0 + sh columns) added after
    phase A."""
    f16 = np.float16
    x = np.asarray(x, np.float32).reshape(IMG, IMG)
    s1 = _build_s1(np.asarray(c1_w, np.float32))
    s2 = _build_s2(np.asarray(c2_w, np.float32))
    s3 = _build_s3(np.asarray(c3_w, np.float32))
    bc3 = np.tile(np.asarray(c3_b, np.float32), 2).reshape(64, 1)

    # phase-B packed consts (same for every core except extrasT)
    cb0 = np.zeros((128, CB16_W), f16)
    w1eT = np.asarray(w1, np.float32)[:, 32768:].T  # [18, 304]
    w2T = np.asarray(w2, np.float32).T  # [304, 176]
    w3T = np.asarray(w3, np.float32).T  # [176, 48]
    cb0[0:18, 512:816] = w1eT
    cb0[0:128, 816:992] = w2T[0:128]
    cb0[0:128, 992:1168] = w2T[128:256]
    cb0[0:48, 1168:1344] = w2T[256:304]
    cb0[0:128, 1344:1392] = w3T[0:128]
    cb0[0:48, 1392:1440] = w3T[128:176]
    shc0 = np.zeros((128, SH_W), np.float32)
    shc0[0:128, 3] = np.asarray(b2, np.float32)[0:128]
    shc0[0:48, 4] = np.asarray(b2, np.float32)[128:176]
    shc0[0:48, 5] = np.asarray(b3, np.float32)

    w1bigT = np.ascontiguousarray(np.asarray(w1, np.float32)[:, :32768].T)  # [32768, 304]
    chunks = x.reshape(G, CS, G, CS).transpose(0, 2, 1, 3).reshape(G * G, CS * CS)
    pi = (np.arange(G * G) // G).astype(np.float32) * CS
    pj = (np.arange(G * G) % G).astype(np.float32) * CS

    P = np.arange(128)
    B = np.arange(32)
    c1b = np.asarray(c1_b, np.float32)
    c2b = np.asarray(c2_b, np.float32)

    maps_a, maps_b = [], []
    for r in range(NCORES):
        cc = np.zeros((96, CC_W), f16)
        # xs: x rows [32r-7, 32r+43), cols padded by 1 each side
        xs = np.zeros((50, 258), f16)
        lo = 32 * r - 7
        hi = 32 * r + 43
        slo, shi = max(lo, 0), min(hi, IMG)
        xs[slo - lo : shi - lo, 1:257] = x[slo:shi, :]
        cc[0:50, 0:258] = xs
        cc[0:18, 258:516] = xs[16:34]
        cc[0:18, 516:900] = s1
        cc[32:50, 516:900] = s1  # duplicate for the base-32 conv1 window
        cc[0:80, 900:1284] = s2
        cc[0:96, 1284:1668] = s3

        ca32 = np.zeros((64, 13), np.float32)
        # row-validity masks (zero out-of-image "phantom" pooled rows)
        for b in range(3):
            for jp in range(8):
                valid = 0 <= (16 * r - 3 + 8 * b + jp) < 128
                ca32[jp * 8 : jp * 8 + 8, 0 + b] = 1.0 if valid else 0.0
                ca32[jp * 8 : jp * 8 + 8, 3 + b] = c1b if valid else 0.0
            for jp in range(4):
                valid = 0 <= (8 * r - 1 + 4 * b + jp) < 64
                ca32[jp * 16 : jp * 16 + 16, 6 + b] = 1.0 if valid else 0.0
                ca32[jp * 16 : jp * 16 + 16, 9 + b] = c2b if valid else 0.0
        ca32[0:64, 12:13] = bc3
        cc[0:64, 1668:1694] = ca32.view(f16)

        # w1ps [128, 32*304]: w1ps[p, j*304+o] = w1[o, kglobal(p, j)],
        # kglobal = (p%32)*1024 + (4r + p//32)*32 + j
        kg = (P[None, :] % 32) * 1024 + (4 * r + P[None, :] // 32) * 32 + B[:, None]
        w1ps = np.ascontiguousarray(
            w1bigT[kg.ravel()]
            .reshape(32, 128, 304)
            .transpose(1, 0, 2)
            .reshape(128, 32 * 304)
            .astype(f16)
        )
        maps_a.append({"cc": cc, "w1ps": w1ps})

        cb = cb0.copy()
        sl = slice(512 * r, 512 * (r + 1))
        cb[0, 0:512] = pi[sl]
        cb[1, 0:512] = pj[sl]
        cb[2:18, 0:512] = chunks[sl].T
        maps_b.append({"cb16": cb})
    return maps_a, maps_b, np.asarray(b1, np.float32), shc0


def _mk_nc():
    import concourse.bacc as bacc

    # Bacc (not raw Bass): its compile() runs move_matmul_waits_to_ldweights /
    # generate_event_semaphores, required for the 1-wait-per-instruction
    # hardware constraint.
    return bacc.Bacc("TRN2", target_bir_lowering=False, debug=False, num_devices=NCORES)


def _warmup(nc, tc, cpool, ppool, f16, f32, n, tag):
    """Dummy matmuls to ramp the PE p-state while DMAs are in flight."""
    wt = cpool.tile([128, 512], f16, tag=f"{tag}wt")
    nc.vector.memset(wt[:], 0.0)
    psw = ppool.tile([1, 512], f32, tag=f"{tag}psw")
    for i in range(n):
        nc.tensor.matmul(
            psw[:],
            lhsT=wt[:, 0:1],
            rhs=wt[:],
            start=(i == 0),
            stop=(i == n - 1),
        )


def _build_phase_a(warm=8):
    """Convs + sharded shared-matvec partial. Output: part [1, 304]."""
    import concourse.tile as tile
    from concourse import mybir

    f32 = mybir.dt.float32
    f16 = mybir.dt.float16
    AF = mybir.ActivationFunctionType
    nc = _mk_nc()

    cc_d = nc.dram_tensor("cc", [96, CC_W], f16, kind="ExternalInput").ap()
    w1ps_d = nc.dram_tensor("w1ps", [128, 32 * 304], f16, kind="ExternalInput").ap()
    part_d = nc.dram_tensor("part", [1, 304], f32, kind="ExternalOutput").ap()

    with tile.TileContext(nc) as tc, ExitStack() as ctx:
        cpool = ctx.enter_context(tc.tile_pool(name="consts", bufs=1))
        spool = ctx.enter_context(tc.tile_pool(name="work", bufs=2))
        pconv = ctx.enter_context(tc.tile_pool(name="pconv", bufs=3, space="PSUM"))
        pmv = ctx.enter_context(tc.tile_pool(name="pmv", bufs=1, space="PSUM"))
        pwrm = ctx.enter_context(tc.tile_pool(name="pwrm", bufs=1, space="PSUM"))

        # warm the ScalarE activation-function table early (overlaps DMAs)
        scr = cpool.tile([1, 1], f32, tag="scr")
        nc.vector.memset(scr[:], 0.0)
        scr2 = cpool.tile([1, 1], f32, tag="scr2")
        nc.scalar.copy(scr2[:], scr[:])
        nc.scalar.activation(scr2[:], scr[:], AF.Relu)

        # all conv consts in ONE DMA job on the Sync queue
        cc_t = cpool.tile([96, CC_W], f16, tag="cc")
        nc.sync.dma_start(cc_t[:], cc_d)

        # w1 stream: 4 chunk DMAs into one [128, 9728] fp16 tile, split
        # across the two HWDGE queues (Sync gets 0,2 after cc; Scalar 1,3)
        wst = cpool.tile([128, 32 * 304], f16, tag="w1s")
        CH = 4
        chw = 32 * 304 // CH
        for c in (1, 3):
            nc.scalar.dma_start(wst[:, c * chw : (c + 1) * chw], w1ps_d[:, c * chw : (c + 1) * chw])
        for c in (0, 2):
            nc.sync.dma_start(wst[:, c * chw : (c + 1) * chw], w1ps_d[:, c * chw : (c + 1) * chw])

        # PE p-state warmup while the DMAs land
        _warmup(nc, tc, cpool, pwrm, f16, f32, warm, "a")

        xs_t = cc_t[0:50, 0:258]
        xs2_t = cc_t[0:18, 258:516]

        def s1ap(dx, base):  # stationary for conv1, at partition base 0 or 32
            return cc_t[base : base + 18, 516 + 128 * dx : 516 + 128 * (dx + 1)]

        def s2ap(dx):
            return cc_t[0:80, 900 + 128 * dx : 900 + 128 * (dx + 1)]

        def s3ap(dx):
            return cc_t[0:96, 1284 + 128 * dx : 1284 + 128 * (dx + 1)]

        ca32_t = cc_t[0:64, 1668:1694].bitcast(f32)  # [64, 13]
        mk1 = lambda b, n=64: ca32_t[0:n, 0 + b : 1 + b]
        bm1 = lambda b, n=64: ca32_t[0:n, 3 + b : 4 + b]
        mk2 = lambda b, n=64: ca32_t[0:n, 6 + b : 7 + b]
        bm2 = lambda b, n=64: ca32_t[0:n, 9 + b : 10 + b]
        bc3 = ca32_t[0:64, 12:13]

        # next-layer moving-window tiles (built in place by ScalarE writes)
        m2 = [cpool.tile([80, 130], f16, tag=f"m2_{i}", name=f"m2_{i}") for i in range(3)]
        m3 = [cpool.tile([96, 66], f16, tag=f"m3_{i}", name=f"m3_{i}") for i in range(2)]
        xc_t = cpool.tile([128, 32], f16, tag="xc")
        for t in m2:
            nc.vector.memset(t[:], 0.0)
        for t in m3:
            nc.vector.memset(t[:], 0.0)

        def pool_to(ps, width):
            """psum [128, width] (m = (s, pair, c)) -> [64, width//2] max-pooled."""
            vtop = spool.tile([64, width], f32, tag=f"vt{width}")
            nc.scalar.copy(vtop[:], ps[0:64, :])
            v = spool.tile([64, width], f32, tag=f"v{width}")
            nc.vector.tensor_max(v[:], ps[64:128, :], vtop[:])
            vv = v[:].rearrange("p (x t) -> p x t", t=2)
            ph = spool.tile([64, width // 2], f32, tag=f"ph{width}")
            nc.vector.tensor_max(ph[:], vv[:, :, 0], vv[:, :, 1])
            return ph

        # ---- conv1: 3 blocks of 16 output rows -> M2 tiles
        win1 = [
            (xs_t[0:18, :], 0),
            (xs2_t, 0),
            (xs_t[32:50, :], 32),
        ]
        for b in range(3):
            rhs, base = win1[b]
            ps = pconv.tile([128, 256], f32, tag="cps")
            for dx in range(3):
                nc.tensor.matmul(
                    ps[:],
                    lhsT=s1ap(dx, base),
                    rhs=rhs[:, dx : dx + 256],
                    start=(dx == 0),
                    stop=(dx == 2),
                )
            ph = pool_to(ps, 256)  # [64, 128]: partition = jp*8+c, row = 8b+jp
            nc.scalar.activation(
                m2[b][0:64, 1:129], ph[:], AF.Relu, bias=bm1(b), scale=mk1(b)
            )
            if b >= 1:  # rows 8b, 8b+1 also tail rows 8..10 of previous window
                nc.scalar.activation(
                    m2[b - 1][64:80, 1:129],
                    ph[0:16, :],
                    AF.Relu,
                    bias=bm1(b, 16),
                    scale=mk1(b, 16),
                )

        # ---- conv2: 3 blocks of 8 output rows -> M3 tiles
        for b in range(3):
            ps = pconv.tile([128, 128], f32, tag="cps")
            for dx in range(3):
                nc.tensor.matmul(
                    ps[:],
                    lhsT=s2ap(dx),
                    rhs=m2[b][:, dx : dx + 128],
                    start=(dx == 0),
                    stop=(dx == 2),
                )
            ph = pool_to(ps, 128)  # [64, 64]: partition = jp'*16+co, row = 4b+jp'
            if b == 0:
                nc.scalar.activation(m3[0][0:64, 1:65], ph[:], AF.Relu, bias=bm2(0), scale=mk2(0))
            elif b == 1:
                nc.scalar.activation(m3[1][0:64, 1:65], ph[:], AF.Relu, bias=bm2(1), scale=mk2(1))
                nc.scalar.activation(
                    m3[0][64:96, 1:65], ph[0:32, :], AF.Relu, bias=bm2(1, 32), scale=mk2(1, 32)
                )
            else:
                nc.scalar.activation(
                    m3[1][64:96, 1:65], ph[0:32, :], AF.Relu, bias=bm2(2, 32), scale=mk2(2, 32)
                )

        # ---- conv3: 2 m-blocks of 4 output rows -> xc [128, 32]
        for g in range(2):
            ps = pconv.tile([128, 64], f32, tag="cps")
            for dx in range(3):
                nc.tensor.matmul(
                    ps[:],
                    lhsT=s3ap(dx),
                    rhs=m3[g][:, dx : dx + 64],
                    start=(dx == 0),
                    stop=(dx == 2),
                )
            ph = pool_to(ps, 64)  # [64, 32]
            nc.scalar.activation(xc_t[64 * g : 64 * g + 64, :], ph[:], AF.Relu, bias=bc3)

        # ---- shared matvec partial [1, 304]
        ps_mv = pmv.tile([1, 304], f32, tag="mv")
        for b in range(32):
            nc.tensor.matmul(
                ps_mv[:],
                lhsT=xc_t[:, b : b + 1],
                rhs=wst[:, 304 * b : 304 * (b + 1)],
                start=(b == 0),
                stop=(b == 31),
            )
        part_s = spool.tile([1, 304], f32, tag="part")
        nc.scalar.copy(part_s[:], ps_mv[:])
        nc.sync.dma_start(part_d, part_s[:])

    nc.compile()
    return nc


def _build_phase_b(warm=6):
    """Patch FC for this core's 512 patches, given summed shared vector."""
    import concourse.tile as tile
    from concourse import mybir

    f32 = mybir.dt.float32
    f16 = mybir.dt.float16
    AF = mybir.ActivationFunctionType
    nc = _mk_nc()

    cb16_d = nc.dram_tensor("cb16", [128, CB16_W], f16, kind="ExternalInput").ap()
    shc_d = nc.dram_tensor("shc", [128, SH_W], f32, kind="ExternalInput").ap()
    yout_d = nc.dram_tensor("yout", [48, 512], f16, kind="ExternalOutput").ap()

    mblk = [(0, 128), (128, 128), (256, 48)]
    qblk = [(0, 128), (128, 48)]

    with tile.TileContext(nc) as tc, ExitStack() as ctx:
        cpool = ctx.enter_context(tc.tile_pool(name="consts", bufs=1))
        fpool = ctx.enter_context(tc.tile_pool(name="fc", bufs=1))
        pfc = ctx.enter_context(tc.tile_pool(name="pfc", bufs=1, space="PSUM"))
        phh = ctx.enter_context(tc.tile_pool(name="phh", bufs=3, space="PSUM"))
        pwrm = ctx.enter_context(tc.tile_pool(name="pwrm", bufs=1, space="PSUM"))

        # warm the ScalarE activation-function table early (overlaps DMAs)
        scr = cpool.tile([1, 1], f32, tag="scr")
        nc.vector.memset(scr[:], 0.0)
        scr2 = cpool.tile([1, 1], f32, tag="scr2")
        nc.scalar.activation(scr2[:], scr[:], AF.Relu)
        nc.scalar.activation(scr2[:], scr[:], AF.Sigmoid)

        cb = cpool.tile([128, CB16_W], f16, tag="cb16")
        # extras + w1eT live in rows 0:18 only — ship just that subrect first
        nc.sync.dma_start(cb[0:18, 0:816], cb16_d[0:18, 0:816])
        nc.sync.dma_start(cb[:, 816:CB16_W], cb16_d[:, 816:CB16_W])
        shc = cpool.tile([128, SH_W], f32, tag="shc")
        nc.scalar.dma_start(shc[:], shc_d)

        # PE p-state warmup while the DMAs land
        _warmup(nc, tc, cpool, pwrm, f16, f32, warm, "b")

        extrasT = cb[0:18, 0:512]
        w1eT = cb[0:18, 512:816]
        w2T_t = [cb[0:128, 816:992], cb[0:128, 992:1168], cb[0:48, 1168:1344]]
        w3T_t = [cb[0:128, 1344:1392], cb[0:48, 1392:1440]]
        sh_t = [shc[0:128, 0:1], shc[0:128, 1:2], shc[0:48, 2:3]]
        b2c_t = [shc[0:128, 3:4], shc[0:48, 4:5]]
        b3c_t = shc[0:48, 5:6]

        from concourse import mybir as _mb

        # h1 = relu(extras@w1e + sh); the three m-blocks go to three
        # different engines so their latency overlaps
        h1_t = []
        h1_eng = [
            lambda h1, ps, sh: nc.scalar.activation(h1, ps, AF.Relu, bias=sh),
            lambda h1, ps, sh: nc.vector.tensor_scalar(
                h1, ps, sh, 0.0, _mb.AluOpType.add, _mb.AluOpType.max
            ),
            lambda h1, ps, sh: nc.vector.tensor_scalar(
                h1, ps, sh, 0.0, _mb.AluOpType.add, _mb.AluOpType.max
            ),
        ]
        for i, (off, mb) in enumerate(mblk):
            ps_e = pfc.tile([mb, 512], f32, tag=f"pse{i}")
            nc.tensor.matmul(
                ps_e[:],
                lhsT=w1eT[:, off : off + mb],
                rhs=extrasT,
                start=True,
                stop=True,
            )
            h1 = fpool.tile([mb, 512], f16, tag=f"h1{i}")
            h1_eng[i](h1[:], ps_e[:], sh_t[i])
            h1_t.append(h1)

        h2_t = []
        for q, (qoff, mq) in enumerate(qblk):
            ps_h = phh.tile([mq, 512], f32, tag="psh")
            for i, (off, mb) in enumerate(mblk):
                nc.tensor.matmul(
                    ps_h[:],
                    lhsT=w2T_t[i][:, qoff : qoff + mq],
                    rhs=h1_t[i][:],
                    start=(i == 0),
                    stop=(i == 2),
                )
            h2 = fpool.tile([mq, 512], f16, tag=f"h2{q}")
            nc.scalar.activation(h2[:], ps_h[:], AF.Relu, bias=b2c_t[q])
            h2_t.append(h2)

        ps_o = phh.tile([48, 512], f32, tag="psh")
        for q, (qoff, mq) in enumerate(qblk):
            nc.tensor.matmul(
                ps_o[:],
                lhsT=w3T_t[q],
                rhs=h2_t[q][:],
                start=(q == 0),
                stop=(q == 1),
            )
        outs = fpool.tile([48, 512], f16, tag="outs")
        nc.scalar.activation(outs[:], ps_o[:], AF.Sigmoid, bias=b3c_t)
        nc.sync.dma_start(yout_d, outs[:])

    nc.compile()
    return nc


def _run(maps_a, maps_b, b1, shc0, trace=False, trace_cores=None):
    from concourse.bass_utils import run_bass_kernel_spmd

    nca = _build_phase_a()
    res_a = run_bass_kernel_spmd(
        nca, maps_a, list(range(NCORES)), trace=trace, trace_cores=trace_cores
    )
    sh = np.sum([res_a.results[r]["part"][0] for r in range(NCORES)], axis=0) + b1
    shc = shc0.copy()
    shc[0:128, 0] = sh[0:128]
    shc[0:128, 1] = sh[128:256]
    shc[0:48, 2] = sh[256:304]
    for mb in maps_b:
        mb["shc"] = shc
    ncb = _build_phase_b()
    res_b = run_bass_kernel_spmd(
        ncb, maps_b, list(range(NCORES)), trace=trace, trace_cores=trace_cores
    )
    full = np.empty((G * G, OUT), np.float32)
    for r in range(NCORES):
        full[512 * r : 512 * (r + 1), :] = res_b.results[r]["yout"].T.astype(np.float32)
    return full.reshape(3, IMG, IMG), res_a, res_b


def kernel(**inputs):
    maps_a, maps_b, b1, shc0 = _host_inputs(**inputs)
    out, _, _ = _run(maps_a, maps_b, b1, shc0)
    return out


if __name__ == "__main__":
    import reference

    inp = {k: np.asarray(v) for k, v in reference.setup_inputs().items()}
    got = kernel(**inp)
    exp = np.asarray(reference.reference(**reference.setup_inputs()))
    err = np.abs(got - exp).max() / max(np.abs(exp).max(), 1e-9)
    print("Relative error:", err)


# revision 7
# speedup vs baseline: 1.5886x; 1.0133x over previous
"""ColorizationNet Trainium2 kernel (8 NeuronCores, SPMD, two phases).

Structure exploited: rows of the big FC input [4096, 32786] share an identical
x_conv prefix (32768 cols), so

    fc_in @ w1.T = x_conv @ w1[:, :32768].T  (one shared matvec, [304])
                 + [pos|chunks] @ w1[:, 32768:].T  ([4096,18] GEMM)

Sharding (core r of 8):
  - conv backbone row-sharded: core r produces the x_conv slice for pooled
    rows [4r, 4r+4) of every channel (halos via zero-padded input windows,
    out-of-image "phantom" rows masked to zero via activation scale).
  - shared matvec K-sharded to match (each core streams 1/8 of w1's big
    part, fp16).  Phase A outputs the 8 partials [304]; the host sums them
    (on-device collectives work here but carry ~50us of cross-core launch
    skew, measured — the host sum between launches is free).
  - phase B: patch FC sharded by patch row, core r handles patches
    [512r, 512(r+1)).

Perf structure (measured on trn2):
  - all matmul operands fp16 (fp32 costs 4 PE cycles/row vs 1; fp16 also
    halves the w1 stream to 2.5 MB/core).  PSUM accumulation stays fp32;
    tolerance is 2e-2, measured error ~6e-4.
  - each dma_start costs ~0.6-0.9us issue + queue latency, so all small
    consts ship as ONE packed fp16 tensor per phase (fp32 mask/bias
    regions ride along bit-cast into fp16 column pairs).
  - the w1 stream is split across the two HWDGE queues (Sync+Scalar);
    a single queue sustains ~390 GB/s only when nothing contends.
  - the PE runs ~2.3x slow until it has been busy ~3us (p-state ramp), so
    both phases issue dummy warmup matmuls during the dead DMA-wait window.
"""

import sys

for _p in ("/opt/trn_rl_repo",):
    if _p not in sys.path:
        sys.path.insert(0, _p)

import numpy as np
from contextlib import ExitStack

IMG = 256
CS = 4
G = 64
H1 = 304
H2 = 176
OUT = 48
NCORES = 8

# phase-A packed const layout, [96, CC_W] fp16:
#   xs [50,258]@0, xs2(=xs rows 16:34) [18,258]@258, s1 [18,384]@516 (+dup
#   rows 32:50), s2 [80,384]@900, s3 [96,384]@1284, ca32 (13 fp32 as 26
#   fp16 cols) [64,26]@1668:  mk1[0:3] bm1[3:6] mk2[6:9] bm2[9:12] bc3[12]
CC_W = 1694
# phase-B fp16 consts, [128, CB16_W]:
#   extrasT [18,512]@0, w1eT [18,304]@512, w2a/b [128,176]@816/@992,
#   w2c [48,176]@1168, w3a [128,48]@1344, w3b [48,48]@1392
CB16_W = 1440
# phase-B fp32 tensor [128, 6]: sh0 sh1 sh2 b2a b2b b3  (sh = summed shared
# vector, packed column-wise after phase A)
SH_W = 6


def _build_s1(c1_w):
    # [18, 3, 128]: rows i = in-row in window; cols m = s*64 + jp*8 + c
    s1 = np.zeros((18, 3, 128), np.float32)
    for dx in range(3):
        for s in range(2):
            for jp in range(8):
                j = 2 * jp + s
                for c in range(8):
                    m = s * 64 + jp * 8 + c
                    for dy in range(3):
                        s1[j + dy, dx, m] = c1_w[c, 0, dy, dx]
    return np.ascontiguousarray(s1.reshape(18, 3 * 128))


def _build_s2(c2_w):
    # [80, 3, 128]: rows k = delta*8 + ci (ci in 0..8); cols m = s*64+jp*16+co
    s2 = np.zeros((80, 3, 128), np.float32)
    for dx in range(3):
        for s in range(2):
            for jp in range(4):
                j2 = 2 * jp + s
                for co in range(16):
                    m = s * 64 + jp * 16 + co
                    for ci in range(8):
                        for dy in range(3):
                            s2[(j2 + dy) * 8 + ci, dx, m] = c2_w[co, ci, dy, dx]
    return np.ascontiguousarray(s2.reshape(80, 3 * 128))


def _build_s3(c3_w):
    # [96, 3, 128]: rows k = delta*16 + ci (ci in 0..16); cols m = s*64+jpp*32+co
    s3 = np.zeros((96, 3, 128), np.float32)
    for dx in range(3):
        for s in range(2):
            for jpp in range(2):
                j3 = 2 * jpp + s
                for co in range(32):
                    m = s * 64 + jpp * 32 + co
                    for ci in range(16):
                        for dy in range(3):
                            s3[(j3 + dy) * 16 + ci, dx, m] = c3_w[co, ci, dy, dx]
    return np.ascontiguousarray(s3.reshape(96, 3 * 128))


def _host_inputs(x, c1_w, c1_b, c2_w, c2_b, c3_w, c3_b, w1, b1, w2, b2, w3, b3):
    """Returns (in_maps_a, in_maps_b_partial, b1, shc0).  Phase-A map:
    'cc' [96,CC_W]f16, 'w1ps' [128,9728]f16.  Phase-B map: 'cb16'
    [128,CB16_W]f16; 'shc' [128,SH_W]f32 (shc0 + sh columns) added after
    phase A."""
    f16 = np.float16
    x = np.asarray(x, np.float32).reshape(IMG, IMG)
    s1 = _build_s1(np.asarray(c1_w, np.float32))
    s2 = _build_s2(np.asarray(c2_w, np.float32))
    s3 = _build_s3(np.asarray(c3_w, np.float32))
    bc3 = np.tile(np.asarray(c3_b, np.float32), 2).reshape(64, 1)

    # phase-B packed consts (same for every core except extrasT)
    cb0 = np.zeros((128, CB16_W), f16)
    w1eT = np.asarray(w1, np.float32)[:, 32768:].T  # [18, 304]
    w2T = np.asarray(w2, np.float32).T  # [304, 176]
    w3T = np.asarray(w3, np.float32).T  # [176, 48]
    cb0[0:18, 512:816] = w1eT
    cb0[0:128, 816:992] = w2T[0:128]
    cb0[0:128, 992:1168] = w2T[128:256]
    cb0[0:48, 1168:1344] = w2T[256:304]
    cb0[0:128, 1344:1392] = w3T[0:128]
    cb0[0:48, 1392:1440] = w3T[128:176]
    shc0 = np.zeros((128, SH_W), np.float32)
    shc0[0:128, 3] = np.asarray(b2, np.float32)[0:128]
    shc0[0:48, 4] = np.asarray(b2, np.float32)[128:176]
    shc0[0:48, 5] = np.asarray(b3, np.float32)

    w1bigT = np.ascontiguousarray(np.asarray(w1, np.float32)[:, :32768].T)  # [32768, 304]
    chunks = x.reshape(G, CS, G, CS).transpose(0, 2, 1, 3).reshape(G * G, CS * CS)
    pi = (np.arange(G * G) // G).astype(np.float32) * CS
    pj = (np.arange(G * G) % G).astype(np.float32) * CS

    P = np.arange(128)
    B = np.arange(32)
    c1b = np.asarray(c1_b, np.float32)
    c2b = np.asarray(c2_b, np.float32)

    maps_a, maps_b = [], []
    for r in range(NCORES):
        cc = np.zeros((96, CC_W), f16)
        # xs: x rows [32r-7, 32r+43), cols padded by 1 each side
        xs = np.zeros((50, 258), f16)
        lo = 32 * r - 7
        hi = 32 * r + 43
        slo, shi = max(lo, 0), min(hi, IMG)
        xs[slo - lo : shi - lo, 1:257] = x[slo:shi, :]
        cc[0:50, 0:258] = xs
        cc[0:18, 258:516] = xs[16:34]
        cc[0:18, 516:900] = s1
        cc[32:50, 516:900] = s1  # duplicate for the base-32 conv1 window
        cc[0:80, 900:1284] = s2
        cc[0:96, 1284:1668] = s3

        ca32 = np.zeros((64, 13), np.float32)
        # row-validity masks (zero out-of-image "phantom" pooled rows)
        for b in range(3):
            for jp in range(8):
                valid = 0 <= (16 * r - 3 + 8 * b + jp) < 128
                ca32[jp * 8 : jp * 8 + 8, 0 + b] = 1.0 if valid else 0.0
                ca32[jp * 8 : jp * 8 + 8, 3 + b] = c1b if valid else 0.0
            for jp in range(4):
                valid = 0 <= (8 * r - 1 + 4 * b + jp) < 64
                ca32[jp * 16 : jp * 16 + 16, 6 + b] = 1.0 if valid else 0.0
                ca32[jp * 16 : jp * 16 + 16, 9 + b] = c2b if valid else 0.0
        ca32[0:64, 12:13] = bc3
        cc[0:64, 1668:1694] = ca32.view(f16)

        # w1ps [128, 32*304]: w1ps[p, j*304+o] = w1[o, kglobal(p, j)],
        # kglobal = (p%32)*1024 + (4r + p//32)*32 + j
        kg = (P[None, :] % 32) * 1024 + (4 * r + P[None, :] // 32) * 32 + B[:, None]
        w1ps = np.ascontiguousarray(
            w1bigT[kg.ravel()]
            .reshape(32, 128, 304)
            .transpose(1, 0, 2)
            .reshape(128, 32 * 304)
            .astype(f16)
        )
        maps_a.append({"cc": cc, "w1ps": w1ps})

        cb = cb0.copy()
        sl = slice(512 * r, 512 * (r + 1))
        cb[0, 0:512] = pi[sl]
        cb[1, 0:512] = pj[sl]
        cb[2:18, 0:512] = chunks[sl].T
        maps_b.append({"cb16": cb})
    return maps_a, maps_b, np.asarray(b1, np.float32), shc0


def _mk_nc():
    import concourse.bacc as bacc

    # Bacc (not raw Bass): its compile() runs move_matmul_waits_to_ldweights /
    # generate_event_semaphores, required for the 1-wait-per-instruction
    # hardware constraint.
    return bacc.Bacc("TRN2", target_bir_lowering=False, debug=False, num_devices=NCORES)


def _warmup(nc, tc, cpool, ppool, f16, f32, n, tag):
    """Dummy matmuls to ramp the PE p-state while DMAs are in flight."""
    wt = cpool.tile([128, 512], f16, tag=f"{tag}wt")
    nc.vector.memset(wt[:], 0.0)
    psw = ppool.tile([1, 512], f32, tag=f"{tag}psw")
    for i in range(n):
        nc.tensor.matmul(
            psw[:],
            lhsT=wt[:, 0:1],
            rhs=wt[:],
            start=(i == 0),
            stop=(i == n - 1),
        )


def _build_phase_a(warm=3):
    """Convs + sharded shared-matvec partial. Output: part [1, 304]."""
    import concourse.tile as tile
    from concourse import mybir

    f32 = mybir.dt.float32
    f16 = mybir.dt.float16
    AF = mybir.ActivationFunctionType
    nc = _mk_nc()

    cc_d = nc.dram_tensor("cc", [96, CC_W], f16, kind="ExternalInput").ap()
    w1ps_d = nc.dram_tensor("w1ps", [128, 32 * 304], f16, kind="ExternalInput").ap()
    part_d = nc.dram_tensor("part", [1, 304], f32, kind="ExternalOutput").ap()

    with tile.TileContext(nc) as tc, ExitStack() as ctx:
        cpool = ctx.enter_context(tc.tile_pool(name="consts", bufs=1))
        spool = ctx.enter_context(tc.tile_pool(name="work", bufs=2))
        pconv = ctx.enter_context(tc.tile_pool(name="pconv", bufs=3, space="PSUM"))
        pmv = ctx.enter_context(tc.tile_pool(name="pmv", bufs=1, space="PSUM"))
        pwrm = ctx.enter_context(tc.tile_pool(name="pwrm", bufs=1, space="PSUM"))

        # warm the ScalarE activation-function table early (overlaps DMAs)
        scr = cpool.tile([1, 1], f32, tag="scr")
        nc.vector.memset(scr[:], 0.0)
        scr2 = cpool.tile([1, 1], f32, tag="scr2")
        nc.scalar.copy(scr2[:], scr[:])
        nc.scalar.activation(scr2[:], scr[:], AF.Relu)

        # all conv consts in ONE DMA job on the Sync queue
        cc_t = cpool.tile([96, CC_W], f16, tag="cc")
        nc.sync.dma_start(cc_t[:], cc_d)

        # w1 stream: 4 chunk DMAs into one [128, 9728] fp16 tile, split
        # across the two HWDGE queues (Sync gets 0,2 after cc; Scalar 1,3)
        wst = cpool.tile([128, 32 * 304], f16, tag="w1s")
        CH = 4
        chw = 32 * 304 // CH
        for c in (1, 3):
            nc.scalar.dma_start(wst[:, c * chw : (c + 1) * chw], w1ps_d[:, c * chw : (c + 1) * chw])
        for c in (0, 2):
            nc.sync.dma_start(wst[:, c * chw : (c + 1) * chw], w1ps_d[:, c * chw : (c + 1) * chw])

        # PE p-state warmup while the DMAs land
        _warmup(nc, tc, cpool, pwrm, f16, f32, warm, "a")

        xs_t = cc_t[0:50, 0:258]
        xs2_t = cc_t[0:18, 258:516]

        def s1ap(dx, base):  # stationary for conv1, at partition base 0 or 32
            return cc_t[base : base + 18, 516 + 128 * dx : 516 + 128 * (dx + 1)]

        def s2ap(dx):
            return cc_t[0:80, 900 + 128 * dx : 900 + 128 * (dx + 1)]

        def s3ap(dx):
            return cc_t[0:96, 1284 + 128 * dx : 1284 + 128 * (dx + 1)]

        ca32_t = cc_t[0:64, 1668:1694].bitcast(f32)  # [64, 13]
        mk1 = lambda b, n=64: ca32_t[0:n, 0 + b : 1 + b]
        bm1 = lambda b, n=64: ca32_t[0:n, 3 + b : 4 + b]
        mk2 = lambda b, n=64: ca32_t[0:n, 6 + b : 7 + b]
        bm2 = lambda b, n=64: ca32_t[0:n, 9 + b : 10 + b]
        bc3 = ca32_t[0:64, 12:13]

        # next-layer moving-window tiles (built in place by ScalarE writes)
        m2 = [cpool.tile([80, 130], f16, tag=f"m2_{i}", name=f"m2_{i}") for i in range(3)]
        m3 = [cpool.tile([96, 66], f16, tag=f"m3_{i}", name=f"m3_{i}") for i in range(2)]
        xc_t = cpool.tile([128, 32], f16, tag="xc")
        for t in m2:
            nc.vector.memset(t[:], 0.0)
        for t in m3:
            nc.vector.memset(t[:], 0.0)

        def pool_to(ps, width):
            """psum [128, width] (m = (s, pair, c)) -> [64, width//2] max-pooled.
            All three ops on VectorE so they run back-to-back (no cross-engine
            semaphore hops); ScalarE only does the trailing activations."""
            vtop = spool.tile([64, width], f32, tag=f"vt{width}")
            nc.vector.tensor_copy(vtop[:], ps[0:64, :])
            v = spool.tile([64, width], f32, tag=f"v{width}")
            nc.vector.tensor_max(v[:], ps[64:128, :], vtop[:])
            vv = v[:].rearrange("p (x t) -> p x t", t=2)
            ph = spool.tile([64, width // 2], f32, tag=f"ph{width}")
            nc.vector.tensor_max(ph[:], vv[:, :, 0], vv[:, :, 1])
            return ph

        # ---- conv1: 3 blocks of 16 output rows -> M2 tiles
        win1 = [
            (xs_t[0:18, :], 0),
            (xs2_t, 0),
            (xs_t[32:50, :], 32),
        ]
        for b in range(3):
            rhs, base = win1[b]
            ps = pconv.tile([128, 256], f32, tag="cps")
            for dx in range(3):
                nc.tensor.matmul(
                    ps[:],
                    lhsT=s1ap(dx, base),
                    rhs=rhs[:, dx : dx + 256],
                    start=(dx == 0),
                    stop=(dx == 2),
                )
            ph = pool_to(ps, 256)  # [64, 128]: partition = jp*8+c, row = 8b+jp
            # tail act FIRST: m2[b-1] completion gates conv2's block b-1
            if b >= 1:  # rows 8b, 8b+1 also tail rows 8..10 of previous window
                nc.scalar.activation(
                    m2[b - 1][64:80, 1:129],
                    ph[0:16, :],
                    AF.Relu,
                    bias=bm1(b, 16),
                    scale=mk1(b, 16),
                )
            nc.scalar.activation(
                m2[b][0:64, 1:129], ph[:], AF.Relu, bias=bm1(b), scale=mk1(b)
            )

        # ---- conv2: 3 blocks of 8 output rows -> M3 tiles
        for b in range(3):
            ps = pconv.tile([128, 128], f32, tag="cps")
            for dx in range(3):
                nc.tensor.matmul(
                    ps[:],
                    lhsT=s2ap(dx),
                    rhs=m2[b][:, dx : dx + 128],
                    start=(dx == 0),
                    stop=(dx == 2),
                )
            ph = pool_to(ps, 128)  # [64, 64]: partition = jp'*16+co, row = 4b+jp'
            if b == 0:
                nc.scalar.activation(m3[0][0:64, 1:65], ph[:], AF.Relu, bias=bm2(0), scale=mk2(0))
            elif b == 1:
                # tail act first: m3[0] completion gates conv3's g=0
                nc.scalar.activation(
                    m3[0][64:96, 1:65], ph[0:32, :], AF.Relu, bias=bm2(1, 32), scale=mk2(1, 32)
                )
                nc.scalar.activation(m3[1][0:64, 1:65], ph[:], AF.Relu, bias=bm2(1), scale=mk2(1))
            else:
                nc.scalar.activation(
                    m3[1][64:96, 1:65], ph[0:32, :], AF.Relu, bias=bm2(2, 32), scale=mk2(2, 32)
                )

        # ---- conv3: 2 m-blocks of 4 output rows -> xc [128, 32]
        for g in range(2):
            ps = pconv.tile([128, 64], f32, tag="cps")
            for dx in range(3):
                nc.tensor.matmul(
                    ps[:],
                    lhsT=s3ap(dx),
                    rhs=m3[g][:, dx : dx + 64],
                    start=(dx == 0),
                    stop=(dx == 2),
                )
            ph = pool_to(ps, 64)  # [64, 32]
            nc.scalar.activation(xc_t[64 * g : 64 * g + 64, :], ph[:], AF.Relu, bias=bc3)

        # ---- shared matvec partial [1, 304]
        ps_mv = pmv.tile([1, 304], f32, tag="mv")
        for b in range(32):
            nc.tensor.matmul(
                ps_mv[:],
                lhsT=xc_t[:, b : b + 1],
                rhs=wst[:, 304 * b : 304 * (b + 1)],
                start=(b == 0),
                stop=(b == 31),
            )
        part_s = spool.tile([1, 304], f32, tag="part")
        nc.scalar.copy(part_s[:], ps_mv[:])
        nc.sync.dma_start(part_d, part_s[:])

    nc.compile()
    return nc


def _build_phase_b(warm=2):
    """Patch FC for this core's 512 patches, given summed shared vector."""
    import concourse.tile as tile
    from concourse import mybir

    f32 = mybir.dt.float32
    f16 = mybir.dt.float16
    AF = mybir.ActivationFunctionType
    nc = _mk_nc()

    cb16_d = nc.dram_tensor("cb16", [128, CB16_W], f16, kind="ExternalInput").ap()
    shc_d = nc.dram_tensor("shc", [128, SH_W], f32, kind="ExternalInput").ap()
    yout_d = nc.dram_tensor("yout", [48, 512], f16, kind="ExternalOutput").ap()

    mblk = [(0, 128), (128, 128), (256, 48)]
    qblk = [(0, 128), (128, 48)]

    with tile.TileContext(nc) as tc, ExitStack() as ctx:
        cpool = ctx.enter_context(tc.tile_pool(name="consts", bufs=1))
        fpool = ctx.enter_context(tc.tile_pool(name="fc", bufs=1))
        pfc = ctx.enter_context(tc.tile_pool(name="pfc", bufs=1, space="PSUM"))
        phh = ctx.enter_context(tc.tile_pool(name="phh", bufs=3, space="PSUM"))
        pwrm = ctx.enter_context(tc.tile_pool(name="pwrm", bufs=1, space="PSUM"))

        # warm the ScalarE activation-function table early (overlaps DMAs)
        scr = cpool.tile([1, 1], f32, tag="scr")
        nc.vector.memset(scr[:], 0.0)
        scr2 = cpool.tile([1, 1], f32, tag="scr2")
        nc.scalar.activation(scr2[:], scr[:], AF.Relu)
        nc.scalar.activation(scr2[:], scr[:], AF.Sigmoid)

        cb = cpool.tile([128, CB16_W], f16, tag="cb16")
        # extras + w1eT live in rows 0:18 only — ship just that subrect first
        nc.sync.dma_start(cb[0:18, 0:816], cb16_d[0:18, 0:816])
        nc.sync.dma_start(cb[:, 816:CB16_W], cb16_d[:, 816:CB16_W])
        shc = cpool.tile([128, SH_W], f32, tag="shc")
        nc.scalar.dma_start(shc[:], shc_d)

        # PE p-state warmup while the DMAs land
        _warmup(nc, tc, cpool, pwrm, f16, f32, warm, "b")

        extrasT = cb[0:18, 0:512]
        w1eT = cb[0:18, 512:816]
        w2T_t = [cb[0:128, 816:992], cb[0:128, 992:1168], cb[0:48, 1168:1344]]
        w3T_t = [cb[0:128, 1344:1392], cb[0:48, 1392:1440]]
        sh_t = [shc[0:128, 0:1], shc[0:128, 1:2], shc[0:48, 2:3]]
        b2c_t = [shc[0:128, 3:4], shc[0:48, 4:5]]
        b3c_t = shc[0:48, 5:6]

        from concourse import mybir as _mb

        # h1 = relu(extras@w1e + sh); the three m-blocks go to three
        # different engines so their latency overlaps
        h1_t = []
        h1_eng = [
            lambda h1, ps, sh: nc.scalar.activation(h1, ps, AF.Relu, bias=sh),
            lambda h1, ps, sh: nc.vector.tensor_scalar(
                h1, ps, sh, 0.0, _mb.AluOpType.add, _mb.AluOpType.max
            ),
            lambda h1, ps, sh: nc.vector.tensor_scalar(
                h1, ps, sh, 0.0, _mb.AluOpType.add, _mb.AluOpType.max
            ),
        ]
        for i, (off, mb) in enumerate(mblk):
            ps_e = pfc.tile([mb, 512], f32, tag=f"pse{i}")
            nc.tensor.matmul(
                ps_e[:],
                lhsT=w1eT[:, off : off + mb],
                rhs=extrasT,
                start=True,
                stop=True,
            )
            h1 = fpool.tile([mb, 512], f16, tag=f"h1{i}")
            h1_eng[i](h1[:], ps_e[:], sh_t[i])
            h1_t.append(h1)

        h2_t = []
        for q, (qoff, mq) in enumerate(qblk):
            ps_h = phh.tile([mq, 512], f32, tag="psh")
            for i, (off, mb) in enumerate(mblk):
                nc.tensor.matmul(
                    ps_h[:],
                    lhsT=w2T_t[i][:, qoff : qoff + mq],
                    rhs=h1_t[i][:],
                    start=(i == 0),
                    stop=(i == 2),
                )
            h2 = fpool.tile([mq, 512], f16, tag=f"h2{q}")
            nc.scalar.activation(h2[:], ps_h[:], AF.Relu, bias=b2c_t[q])
            h2_t.append(h2)

        ps_o = phh.tile([48, 512], f32, tag="psh")
        for q, (qoff, mq) in enumerate(qblk):
            nc.tensor.matmul(
                ps_o[:],
                lhsT=w3T_t[q],
                rhs=h2_t[q][:],
                start=(q == 0),
                stop=(q == 1),
            )
        outs = fpool.tile([48, 512], f16, tag="outs")
        nc.scalar.activation(outs[:], ps_o[:], AF.Sigmoid, bias=b3c_t)
        nc.sync.dma_start(yout_d, outs[:])

    nc.compile()
    return nc


def _run(maps_a, maps_b, b1, shc0, trace=False, trace_cores=None):
    from concourse.bass_utils import run_bass_kernel_spmd

    nca = _build_phase_a()
    res_a = run_bass_kernel_spmd(
        nca, maps_a, list(range(NCORES)), trace=trace, trace_cores=trace_cores
    )
    sh = np.sum([res_a.results[r]["part"][0] for r in range(NCORES)], axis=0) + b1
    shc = shc0.copy()
    shc[0:128, 0] = sh[0:128]
    shc[0:128, 1] = sh[128:256]
    shc[0:48, 2] = sh[256:304]
    for mb in maps_b:
        mb["shc"] = shc
    ncb = _build_phase_b()
    res_b = run_bass_kernel_spmd(
        ncb, maps_b, list(range(NCORES)), trace=trace, trace_cores=trace_cores
    )
    full = np.empty((G * G, OUT), np.float32)
    for r in range(NCORES):
        full[512 * r : 512 * (r + 1), :] = res_b.results[r]["yout"].T.astype(np.float32)
    return full.reshape(3, IMG, IMG), res_a, res_b


def kernel(**inputs):
    maps_a, maps_b, b1, shc0 = _host_inputs(**inputs)
    out, _, _ = _run(maps_a, maps_b, b1, shc0)
    return out


if __name__ == "__main__":
    import reference

    inp = {k: np.asarray(v) for k, v in reference.setup_inputs().items()}
    got = kernel(**inp)
    exp = np.asarray(reference.reference(**reference.setup_inputs()))
    err = np.abs(got - exp).max() / max(np.abs(exp).max(), 1e-9)
    print("Relative error:", err)
